# revision 4
# baseline (speedup 1.0000x reference)
"""DeepLSTM (3-layer, skip-connection) Trainium2 kernel, 8-way tensor-parallel.

Sharding: core k owns hidden slice [k*128,(k+1)*128) of every layer (gate
columns i/f/o/j for that slice). Recurrent weights stay SBUF-resident.
Phase A precomputes Ax[l,t] = x_t @ Wx_l + b_l (gate-slice) into DRAM.
Phase B runs a skewed wavefront (layer l at t = w - l), one AllGather of
bf16 h-slices (transposed to [hid,batch]) per wave across the 8 cores.
"""
import numpy as np
import ml_dtypes

import concourse.bass as bass
import concourse.bacc as bacc
import concourse.mybir as mybir
import concourse.tile as tile
from concourse.bass import IndirectOffsetOnAxis
from concourse.bass_utils import run_bass_kernel_spmd

BF16 = mybir.dt.bfloat16
F32 = mybir.dt.float32
I32 = mybir.dt.int32

VOCAB, SIZE, DEPTH, BATCH = 32000, 1024, 3, 64
NCORE = 8
NCHUNK = SIZE // 128          # 8 K-chunks of the hidden dim
GSL = 4 * SIZE // NCORE       # 512 gate columns per core
AF = mybir.ActivationFunctionType

bf16 = ml_dtypes.bfloat16

# measured logical->physical NC map on this machine (probe_map.py); the
# remote_dma XOR-delta routing acts on PHYSICAL tpb indices
PHYS_MAP = [0, 1, 2, 3, 6, 7, 4, 5]


def build(T, interleave=True, reps=1, pack2=False,
          ag_mode="cc", phase_a=True, comm="cc"):
    """Build the SPMD bass program for T timesteps. Returns nc.

    reps > 1 repeats the whole computation back-to-back (same inputs,
    state re-zeroed) — used to measure pure HW exec time by differencing.
    """
    n_tiles = (T * BATCH) // 128          # token tiles of 128 (2 timesteps each)
    assert (T * BATCH) % 128 == 0
    n_waves = T + DEPTH - 1               # skewed wavefront

    nc = bacc.Bacc("TRN2", target_bir_lowering=False, debug=False,
                   num_devices=NCORE,
                   detect_race_conditions=(comm != "rdma"))

    # ---- DRAM I/O ----
    tok = nc.dram_tensor("tok", [128, n_tiles], I32, kind="ExternalInput")
    embt = nc.dram_tensor("embt", [VOCAB, SIZE], BF16, kind="ExternalInput")
    wx = nc.dram_tensor("wx", [128, NCHUNK, DEPTH, GSL], BF16, kind="ExternalInput")
    wr = nc.dram_tensor("wr", [128, DEPTH, 2 * NCHUNK, GSL], BF16, kind="ExternalInput")
    bias = nc.dram_tensor("bias", [1, DEPTH, GSL], BF16, kind="ExternalInput")
    idbf = nc.dram_tensor("idbf", [128, 128], BF16, kind="ExternalInput")
    idf32 = nc.dram_tensor("idf32", [64, 64], F32, kind="ExternalInput")
    ones = nc.dram_tensor("ones", [1, 128], BF16, kind="ExternalInput")
    out = nc.dram_tensor("out", [DEPTH, BATCH, 128], F32, kind="ExternalOutput")

    ax_dram = nc.dram_tensor("ax_dram", [DEPTH, n_tiles, 128, GSL], BF16)

    rdma = None
    if comm == "rdma":
        rdma = {
            "rsem": nc.alloc_semaphore("rdma_recv"),
            "lsem": nc.alloc_semaphore("rdma_sent"),
            "sent_waves": 0,   # cumulative across reps
            "patches": [],     # (mybir inst, rsem target) applied post-Tile
        }

    if rdma is not None:
        # all cores must be inside this NEFF before any remote SBUF write
        nc.gpsimd.bir_kernel_barrier_wait([list(range(NCORE))])

    with tile.TileContext(nc) as tc:
        with (
            tc.tile_pool(name="const", bufs=1) as constp,
            tc.tile_pool(name="state", bufs=1) as statep,
            tc.tile_pool(name="psum", bufs=1, space="PSUM") as psum,
            tc.tile_pool(name="work", bufs=2) as work,
            tc.tile_pool(name="dram", bufs=2, space="DRAM") as dram,
        ):
            # ---- resident SBUF constants/weights ----
            tok_sb = constp.tile([128, n_tiles], I32)
            nc.sync.dma_start(out=tok_sb[:], in_=tok[:])
            wx_sb = constp.tile([128, NCHUNK, DEPTH, GSL], BF16)
            nc.sync.dma_start(out=wx_sb[:], in_=wx[:])
            wr_sb = constp.tile([128, DEPTH, 2 * NCHUNK, GSL], BF16)
            nc.sync.dma_start(out=wr_sb[:], in_=wr[:])
            bias_sb = constp.tile([1, DEPTH, GSL], BF16)
            nc.sync.dma_start(out=bias_sb[:], in_=bias[:])
            idbf_sb = constp.tile([128, 128], BF16)
            nc.sync.dma_start(out=idbf_sb[:], in_=idbf[:])
            idf32_sb = constp.tile([64, 64], F32)
            nc.sync.dma_start(out=idf32_sb[:], in_=idf32[:])
            idf32_hi = constp.tile([128, 64], F32)
            nc.sync.dma_start(out=idf32_hi[64:128, :], in_=idf32[:])
            ones_sb = constp.tile([1, 128], BF16)
            nc.sync.dma_start(out=ones_sb[:], in_=ones[:])

            # ---- state ----
            c_st = statep.tile([128, DEPTH, 128], F32)  # cell state (own slice)
            hT_zero = statep.tile([128, DEPTH, NCHUNK, 64], BF16)
            nc.gpsimd.memset(hT_zero[:], 0.0)
            ztile = statep.tile([128, 64], BF16)
            nc.gpsimd.memset(ztile[:], 0.0)
            if rdma is not None:
                # double-buffered (by wave parity) send/recv SBUF buffers for
                # the hand-rolled all-gather via remote SBUF->SBUF DMA
                rdma["hstg"] = statep.tile([128, 2, DEPTH * 64], BF16,
                                           name="rdma_hstg")
                rdma["hTr"] = statep.tile([128, 2, NCORE, DEPTH, 64], BF16,
                                          name="rdma_hTr")

            for rep in range(reps):
                emit_rep(nc, tc, psum, work, dram, T, n_tiles, n_waves,
                         interleave, rep,
                         tok_sb, wx_sb, wr_sb, bias_sb, idbf_sb, idf32_sb,
                         ones_sb, c_st, hT_zero, ztile, ax_dram, embt, out,
                         pack2, idf32_hi, ag_mode, phase_a, rdma)

    if rdma is not None:
        # Inject arrival waits on every gathered-h consumer AFTER Tile
        # scheduling (the scheduler's single-core sim cannot model remote
        # semaphore increments and would deadlock on explicit wait_ge).
        sem_id = rdma["rsem"].num
        for ins, target in rdma["patches"]:
            wsync = mybir.SyncWait(sync_type="semaphore", id=sem_id,
                                   wait_mode="sem-ge-imm", wait_value=target,
                                   ant_name="rdma_recv")
            si = ins.sync_info
            if si is None:
                ins.sync_info = mybir.SyncInfo(on_wait=[wsync], on_update=[])
            else:
                si.on_wait = list(si.on_wait) + [wsync]
                ins.sync_info = si
    nc.compile()
    return nc


def emit_rep(nc, tc, psum, work, dram, T, n_tiles, n_waves, interleave, rep,
             tok_sb, wx_sb, wr_sb, bias_sb, idbf_sb, idf32_sb, ones_sb,
             c_st, hT_zero, ztile, ax_dram, embt, out, pack2=False,
             idf32_hi=None, ag_mode="cc", phase_a=True, rdma=None):
    if True:
        if True:
            nc.gpsimd.memset(c_st[:], 0.0)
            rp = f"r{rep}_"

            # ---- phase A: one token tile (128 tokens = 2 steps) ----
            def emit_phaseA_tile(i):
                xg = work.tile([128, SIZE], BF16, tag="xg", bufs=2,
                               name=f"{rp}xg{i}")
                nc.gpsimd.indirect_dma_start(
                    out=xg[:], out_offset=None, in_=embt[:, :],
                    in_offset=IndirectOffsetOnAxis(ap=tok_sb[:, i:i + 1], axis=0),
                )
                xt_ps = psum.tile([128, SIZE], BF16, tag="xt_ps", bufs=1,
                                  name=f"{rp}xtps{i}")
                for ch in range(NCHUNK):
                    nc.tensor.transpose(
                        out=xt_ps[:, ch * 128:(ch + 1) * 128],
                        in_=xg[:, ch * 128:(ch + 1) * 128],
                        identity=idbf_sb[:],
                    )
                xt = work.tile([128, SIZE], BF16, tag="xt", bufs=2, name=f"{rp}xt{i}")
                nc.vector.tensor_copy(xt[:], xt_ps[:])
                for l in range(DEPTH):
                    ax_ps = psum.tile([128, GSL], F32, tag="ax_ps", bufs=1,
                                      name=f"{rp}axps{i}_{l}")
                    for ch in range(NCHUNK):
                        nc.tensor.matmul(
                            ax_ps[:], lhsT=xt[:, ch * 128:(ch + 1) * 128],
                            rhs=wx_sb[:, ch, l, :],
                            start=(ch == 0), stop=False)
                    nc.tensor.matmul(ax_ps[:], lhsT=ones_sb[:, :],
                                     rhs=bias_sb[:, l, :], start=False, stop=True)
                    ax_sb = work.tile([128, GSL], BF16, tag="ax_st", bufs=3,
                                      name=f"{rp}axsb{i}_{l}")
                    nc.scalar.copy(ax_sb[:], ax_ps[:])
                    nc.sync.dma_start(out=ax_dram[l, i, :, :], in_=ax_sb[:])

            n_emitted = [0]

            def ensure_phaseA(upto):
                if not phase_a:
                    return  # timing-only variant: skip Ax precompute
                while n_emitted[0] < min(upto, n_tiles):
                    emit_phaseA_tile(n_emitted[0])
                    n_emitted[0] += 1

            if not interleave:
                ensure_phaseA(n_tiles)

            # ---- phase B: wavefront ----
            from concourse.bass import _add_dep_helper
            hT_cur = hT_zero
            for w in range(n_waves):
                if interleave:
                    # keep ~4 tiles of lookahead over the consuming wave
                    ensure_phaseA(w // 2 + 5)
                if rdma is None:
                    cin = dram.tile([DEPTH, 128, 64], BF16, tag="cin", bufs=2,
                                    name=f"{rp}cin{w}")
                    for l in range(DEPTH):
                        if not (0 <= w - l < T) and w < n_waves - 1:
                            # inactive layer: keep its gather slice zero/finite
                            nc.sync.dma_start(out=cin[l, :, :], in_=ztile[:])
                    def lhs_chunk(l, ch):
                        return hT_cur[:, l, ch, :]
                else:
                    wp = w % 2
                    if w == 0:
                        def lhs_chunk(l, ch):
                            return hT_zero[:, l, ch, :]
                    else:
                        def lhs_chunk(l, ch, _p=(w - 1) % 2):
                            return rdma["hTr"][:, _p, ch, l, :]
                actives = [l for l in range(DEPTH) if 0 <= w - l < T]
                ginfo = {}
                for l in actives:
                    t = w - l
                    ax_sb = work.tile([64, GSL], BF16, tag="ax_ld", bufs=6,
                                      name=f"{rp}axld{w}_{l}")
                    nc.sync.dma_start(
                        out=ax_sb[:],
                        in_=ax_dram[l, t // 2, (t % 2) * 64:(t % 2) * 64 + 64, :])
                    # pack2: layer 2 runs in PE array cols 64:128 concurrently
                    # with layer 1 in cols 0:64 (own psum bank, partitions
                    # 64:128); its whole elementwise chain stays on those
                    # partitions.
                    pb = 64 if (pack2 and l == 2) else 0
                    if pb:
                        g_full = psum.tile([128, GSL], F32, tag="g2", bufs=2,
                                           name=f"{rp}g{w}_{l}")
                        gv = g_full[64:128, :]
                        tp = (0, 64)
                    else:
                        gbufs = 2 if pack2 else 3
                        g_full = psum.tile([64, GSL], F32, tag="g", bufs=gbufs,
                                           name=f"{rp}g{w}_{l}")
                        gv = g_full[:]
                        tp = None
                    # inject Ax (+bias) via identity matmul, clears the bank
                    nc.tensor.matmul(gv, lhsT=idbf_sb[:64, :64],
                                     rhs=ax_sb[:],
                                     start=True, stop=False,
                                     tile_position=tp)
                    mms = [(lhs_chunk(l, ch), wr_sb[:, l, NCHUNK + ch, :])
                           for ch in range(NCHUNK)]
                    if l > 0:                  # prev-layer skip input @ Wp
                        mms += [(lhs_chunk(l - 1, ch), wr_sb[:, l, ch, :])
                                for ch in range(NCHUNK)]
                    ginfo[l] = (t, pb, gv, tp, mms)

                def emit_stream(l, j):
                    t, pb, gv, tp, mms = ginfo[l]
                    lhsT, rhs = mms[j]
                    bi = nc.tensor.matmul(gv, lhsT=lhsT, rhs=rhs, start=False,
                                          stop=(j == len(mms) - 1),
                                          tile_position=tp)
                    if rdma is not None and w > 0:
                        rdma["patches"].append(
                            (bi.ins, 14 * rdma["sent_waves"]))

                if pack2:
                    paired = [l for l in (1, 2) if l in ginfo]
                    plen = max((len(ginfo[l][4]) for l in paired), default=0)
                    for j in range(plen):
                        for l in paired:
                            if j < len(ginfo[l][4]):
                                emit_stream(l, j)
                    if 0 in ginfo:
                        for j in range(len(ginfo[0][4])):
                            emit_stream(0, j)
                else:
                    for l in actives:
                        for j in range(len(ginfo[l][4])):
                            emit_stream(l, j)

                for l in actives:
                    t, pb, gv, tp, mms = ginfo[l]
                    sl = slice(pb, pb + 64)
                    cs = c_st[sl, l, :]
                    # gates: layout [i|f|o|j], 128 each
                    sifo = work.tile([128, 384], F32, tag="sifo", bufs=3,
                                     name=f"{rp}sifo{w}_{l}")
                    nc.scalar.activation(sifo[sl, :], gv[:, 0:384], AF.Sigmoid)
                    tj = work.tile([128, 128], F32, tag="tj", bufs=3,
                                   name=f"{rp}tj{w}_{l}")
                    nc.scalar.activation(tj[sl, :], gv[:, 384:512], AF.Tanh)
                    t1 = work.tile([128, 128], F32, tag="t1", bufs=3,
                                   name=f"{rp}t1{w}_{l}")
                    nc.vector.tensor_mul(t1[sl, :], sifo[sl, 0:128], tj[sl, :])
                    t2 = work.tile([128, 128], F32, tag="t2", bufs=3,
                                   name=f"{rp}t2{w}_{l}")
                    nc.vector.tensor_mul(t2[sl, :], cs, sifo[sl, 128:256])
                    nc.vector.tensor_add(cs, t1[sl, :], t2[sl, :])
                    th = work.tile([128, 128], F32, tag="th", bufs=3,
                                   name=f"{rp}th{w}_{l}")
                    nc.scalar.activation(th[sl, :], cs, AF.Tanh)
                    hnew = work.tile([128, 128], F32, tag="hnew", bufs=3,
                                     name=f"{rp}hnew{w}_{l}")
                    nc.vector.tensor_mul(hnew[sl, :], th[sl, :], sifo[sl, 256:384])
                    if t == T - 1:
                        nc.sync.dma_start(out=out[l, :, :], in_=hnew[sl, :])
                        if l == DEPTH - 1:
                            continue  # top layer: no future consumer
                    ht_ps = psum.tile([128, 64], F32, tag="htp", bufs=2,
                                      name=f"{rp}htp{w}_{l}")
                    ident = idf32_hi[64:128, :] if pb else idf32_sb[:]
                    nc.tensor.transpose(ht_ps[:], hnew[sl, :], identity=ident,
                                        tile_position=(pb, 0))
                    if rdma is None:
                        hstg = work.tile([128, 64], BF16, tag="hstg", bufs=3,
                                         name=f"{rp}hstg{w}_{l}")
                        nc.vector.tensor_copy(hstg[:], ht_ps[:])
                        nc.sync.dma_start(out=cin[l, :, :], in_=hstg[:])
                    else:
                        nc.vector.tensor_copy(
                            rdma["hstg"][:, wp, l * 64:(l + 1) * 64], ht_ps[:])
                if w >= n_waves - 1:
                    continue  # last wave: no gather needed
                if rdma is not None:
                    # zero inactive-layer slices of the send buffer (finite +
                    # correct zeros for the t<0 self-inputs of early waves)
                    for l in range(DEPTH):
                        if not (0 <= w - l < T) and w < 2:
                            nc.vector.tensor_copy(
                                rdma["hstg"][:, wp, l * 64:(l + 1) * 64],
                                ztile[:])
                    hsrc = rdma["hstg"][:, wp, :]
                    # own slice lands in slot 0 (r ^ 0 = r)
                    nc.vector.tensor_copy(
                        rdma["hTr"][:, wp, 0, :, :],
                        hsrc.rearrange("p (l b) -> p l b", l=DEPTH))
                    for d in range(1, NCORE):
                        nc.gpsimd.remote_dma_broadcast(
                            out_ap=rdma["hTr"][:, wp, d, :, :],
                            in_ap=hsrc,
                            remote_sem=rdma["rsem"],
                            local_sem=rdma["lsem"],
                            rdests=[(0, d) if k == d else None
                                    for k in range(NCORE)],
                        )
                    nc.gpsimd.trigger_dma(count=None)
                    rdma["sent_waves"] += 1
                    continue
                cout = dram.tile([NCORE, DEPTH, 128, 64], BF16, tag="cout",
                                 bufs=2, addr_space="Shared", name=f"{rp}cout{w}")
                if ag_mode == "cc":
                    nc.gpsimd.collective_compute(
                        "AllGather", mybir.AluOpType.bypass,
                        replica_groups=[list(range(NCORE))],
                        ins=[cin.opt()], outs=[cout.opt()],
                    )
                else:  # timing-only variant: local copy instead of AllGather
                    nc.sync.dma_start(out=cout[0, :, :, :], in_=cin[:, :, :])
                hT_g = work.tile([128, DEPTH, NCHUNK, 64], BF16, tag="hTg",
                                 bufs=2, name=f"{rp}hTg{w}")
                for l in range(DEPTH):
                    nc.sync.dma_start(
                        out=hT_g[:, l, :, :],
                        in_=cout[:, l, :, :].rearrange("r p b -> p r b"))
                hT_cur = hT_g
            ensure_phaseA(n_tiles)  # no-op normally


# ---------------- v2: split-gate dual-column-group wavefront ----------------

def build_v2(T, reps=1, comm="rdma", interleave=True, phase_a=True):
    """v2: each layer's 512 gate cols (reordered [i0 f0 o0 j0 | i1 f1 o1 j1]
    by hidden half) are computed by two concurrent PE column-group streams
    (batch=64 rows each, N=256), landing in one [128, 256] PSUM tile
    (partitions = batch x half). Elementwise then runs full-width [128, *].
    """
    n_tiles = (T * BATCH) // 128
    assert (T * BATCH) % 128 == 0
    n_waves = T + DEPTH - 1

    nc = bacc.Bacc("TRN2", target_bir_lowering=False, debug=False,
                   num_devices=NCORE,
                   detect_race_conditions=(comm != "rdma"))

    tok = nc.dram_tensor("tok", [128, n_tiles], I32, kind="ExternalInput")
    embt = nc.dram_tensor("embt", [VOCAB, SIZE], BF16, kind="ExternalInput")
    wx = nc.dram_tensor("wx", [128, NCHUNK, DEPTH, GSL], BF16, kind="ExternalInput")
    wr = nc.dram_tensor("wr", [128, DEPTH, 2 * NCHUNK, GSL], BF16, kind="ExternalInput")
    bias = nc.dram_tensor("bias", [1, DEPTH, GSL], BF16, kind="ExternalInput")
    idbf = nc.dram_tensor("idbf", [128, 128], BF16, kind="ExternalInput")
    idf32 = nc.dram_tensor("idf32", [64, 64], F32, kind="ExternalInput")
    ones = nc.dram_tensor("ones", [1, 128], BF16, kind="ExternalInput")
    out = nc.dram_tensor("out", [DEPTH, 128, 64], F32, kind="ExternalOutput")

    ax_dram = nc.dram_tensor("ax_dram", [DEPTH, n_tiles, 128, GSL], BF16)

    rdma = None
    if comm == "rdma":
        rdma = {
            "rsem": nc.alloc_semaphore("rdma_recv"),
            "lsem": nc.alloc_semaphore("rdma_sent"),
            "sent_waves": 0,
            "patches": [],
        }
        nc.gpsimd.bir_kernel_barrier_wait([list(range(NCORE))])

    with tile.TileContext(nc) as tc:
        with (
            tc.tile_pool(name="const", bufs=1) as constp,
            tc.tile_pool(name="state", bufs=1) as statep,
            tc.tile_pool(name="psum", bufs=1, space="PSUM") as psum,
            tc.tile_pool(name="work", bufs=2) as work,
            tc.tile_pool(name="dram", bufs=2, space="DRAM") as dram,
        ):
            tok_sb = constp.tile([128, n_tiles], I32)
            nc.sync.dma_start(out=tok_sb[:], in_=tok[:])
            wx_sb = constp.tile([128, NCHUNK, DEPTH, GSL], BF16)
            nc.sync.dma_start(out=wx_sb[:], in_=wx[:])
            wr_sb = constp.tile([128, DEPTH, 2 * NCHUNK, GSL], BF16)
            nc.sync.dma_start(out=wr_sb[:], in_=wr[:])
            bias_sb = constp.tile([1, DEPTH, GSL], BF16)
            nc.sync.dma_start(out=bias_sb[:], in_=bias[:])
            idbf_sb = constp.tile([128, 128], BF16)
            nc.sync.dma_start(out=idbf_sb[:], in_=idbf[:])
            idf32_sb = constp.tile([64, 64], F32)
            nc.sync.dma_start(out=idf32_sb[:], in_=idf32[:])
            idf32_hi = constp.tile([128, 64], F32)
            nc.sync.dma_start(out=idf32_hi[64:128, :], in_=idf32[:])
            ones_sb = constp.tile([1, 128], BF16)
            nc.sync.dma_start(out=ones_sb[:], in_=ones[:])

            # state: partitions = batch (0:64 -> hid half 0, 64:128 -> half 1)
            c_st = statep.tile([128, DEPTH, 64], F32)
            hT_zero = statep.tile([128, DEPTH, NCHUNK, 64], BF16)
            nc.gpsimd.memset(hT_zero[:], 0.0)
            ztile = statep.tile([128, 64], BF16)
            nc.gpsimd.memset(ztile[:], 0.0)
            if rdma is not None:
                rdma["hstg"] = statep.tile([128, 2, DEPTH * 64], BF16,
                                           name="rdma_hstg")
                rdma["hTr"] = statep.tile([128, 2, NCORE, DEPTH, 64], BF16,
                                          name="rdma_hTr")

            for rep in range(reps):
                emit_rep_v2(nc, tc, psum, work, dram, T, n_tiles, n_waves,
                            interleave, rep,
                            tok_sb, wx_sb, wr_sb, bias_sb, idbf_sb, idf32_sb,
                            ones_sb, c_st, hT_zero, ztile, ax_dram, embt, out,
                            idf32_hi, phase_a, rdma)

    if rdma is not None:
        sem_id = rdma["rsem"].num
        for ins, target in rdma["patches"]:
            wsync = mybir.SyncWait(sync_type="semaphore", id=sem_id,
                                   wait_mode="sem-ge-imm", wait_value=target,
                                   ant_name="rdma_recv")
            si = ins.sync_info
            if si is None:
                ins.sync_info = mybir.SyncInfo(on_wait=[wsync], on_update=[])
            else:
                si.on_wait = list(si.on_wait) + [wsync]
                ins.sync_info = si
    nc.compile()
    return nc


def emit_rep_v2(nc, tc, psum, work, dram, T, n_tiles, n_waves, interleave, rep,
                tok_sb, wx_sb, wr_sb, bias_sb, idbf_sb, idf32_sb, ones_sb,
                c_st, hT_zero, ztile, ax_dram, embt, out,
                idf32_hi, phase_a=True, rdma=None):
    nc.gpsimd.memset(c_st[:], 0.0)
    rp = f"r{rep}_"

    # ---- phase A (identical mechanics to v1; gate order is host-side) ----
    def emit_phaseA_tile(i):
        xg = work.tile([128, SIZE], BF16, tag="xg", bufs=2, name=f"{rp}xg{i}")
        nc.gpsimd.indirect_dma_start(
            out=xg[:], out_offset=None, in_=embt[:, :],
            in_offset=IndirectOffsetOnAxis(ap=tok_sb[:, i:i + 1], axis=0),
        )
        xt_ps = psum.tile([128, SIZE], BF16, tag="xt_ps", bufs=1,
                          name=f"{rp}xtps{i}")
        for ch in range(NCHUNK):
            nc.tensor.transpose(
                out=xt_ps[:, ch * 128:(ch + 1) * 128],
                in_=xg[:, ch * 128:(ch + 1) * 128],
                identity=idbf_sb[:],
            )
        xt = work.tile([128, SIZE], BF16, tag="xt", bufs=2, name=f"{rp}xt{i}")
        nc.vector.tensor_copy(xt[:], xt_ps[:])
        for l in range(DEPTH):
            ax_ps = psum.tile([128, GSL], F32, tag="ax_ps", bufs=1,
                              name=f"{rp}axps{i}_{l}")
            for ch in range(NCHUNK):
                nc.tensor.matmul(
                    ax_ps[:], lhsT=xt[:, ch * 128:(ch + 1) * 128],
                    rhs=wx_sb[:, ch, l, :],
                    start=(ch == 0), stop=False)
            nc.tensor.matmul(ax_ps[:], lhsT=ones_sb[:, :],
                             rhs=bias_sb[:, l, :], start=False, stop=True)
            ax_sb = work.tile([128, GSL], BF16, tag="ax_st", bufs=3,
                              name=f"{rp}axsb{i}_{l}")
            nc.scalar.copy(ax_sb[:], ax_ps[:])
            nc.sync.dma_start(out=ax_dram[l, i, :, :], in_=ax_sb[:])

    n_emitted = [0]

    def ensure_phaseA(upto):
        if not phase_a:
            return
        while n_emitted[0] < min(upto, n_tiles):
            emit_phaseA_tile(n_emitted[0])
            n_emitted[0] += 1

    if not interleave:
        ensure_phaseA(n_tiles)

    # ---- phase B ----
    hT_cur = hT_zero
    for w in range(n_waves):
        if interleave:
            ensure_phaseA(w // 2 + 5)
        if rdma is None:
            cin = dram.tile([DEPTH, 128, 64], BF16, tag="cin", bufs=2,
                            name=f"{rp}cin{w}")
            for l in range(DEPTH):
                if not (0 <= w - l < T) and w < n_waves - 1:
                    nc.sync.dma_start(out=cin[l, :, :], in_=ztile[:])
            def lhs_chunk(l, ch):
                return hT_cur[:, l, ch, :]
        else:
            wp = w % 2
            if w == 0:
                def lhs_chunk(l, ch):
                    return hT_zero[:, l, ch, :]
            else:
                def lhs_chunk(l, ch, _p=(w - 1) % 2):
                    return rdma["hTr"][:, _p, ch, l, :]
        actives = [l for l in range(DEPTH) if 0 <= w - l < T]
        ginfo = {}
        for l in actives:
            t = w - l
            ax_sb = work.tile([64, GSL], BF16, tag="ax_ld", bufs=6,
                              name=f"{rp}axld{w}_{l}")
            nc.sync.dma_start(
                out=ax_sb[:],
                in_=ax_dram[l, t // 2, (t % 2) * 64:(t % 2) * 64 + 64, :])
            # one [128, 256] psum tile per layer; padded to a full bank so
            # no two accumulation groups share a bank (has_written clear is
            # bank-wide). Stream A -> partitions 0:64 (cols 0:256 of the
            # reordered gates), stream B -> partitions 64:128 (cols 256:512).
            g = psum.tile([128, 256], F32, tag="g", bufs=3,
                          padded_shape=[128, 512], name=f"{rp}g{w}_{l}")
            # inject Ax+bias via identity matmuls (clears both halves)
            nc.tensor.matmul(g[0:64, :], lhsT=idbf_sb[:64, :64],
                             rhs=ax_sb[:, 0:256], start=True, stop=False,
                             tile_position=(0, 0))
            nc.tensor.matmul(g[64:128, :], lhsT=idbf_sb[:64, :64],
                             rhs=ax_sb[:, 256:512], start=True, stop=False,
                             tile_position=(0, 64))
            mms = [(lhs_chunk(l, ch), wr_sb[:, l, NCHUNK + ch, :])
                   for ch in range(NCHUNK)]
            if l > 0:
                mms += [(lhs_chunk(l - 1, ch), wr_sb[:, l, ch, :])
                        for ch in range(NCHUNK)]
            ginfo[l] = (t, g, mms)

        for l in actives:
            t, g, mms = ginfo[l]
            for j, (lhsT, rhs) in enumerate(mms):
                last = (j == len(mms) - 1)
                biA = nc.tensor.matmul(g[0:64, :], lhsT=lhsT, rhs=rhs[..., 0:256],
                                       start=False, stop=last,
                                       tile_position=(0, 0))
                biB = nc.tensor.matmul(g[64:128, :], lhsT=lhsT,
                                       rhs=rhs[..., 256:512],
                                       start=False, stop=last,
                                       tile_position=(0, 64))
                if rdma is not None and w > 0:
                    tgt = 14 * rdma["sent_waves"]
                    rdma["patches"].append((biA.ins, tgt))
                    rdma["patches"].append((biB.ins, tgt))

        for l in actives:
            t, g, mms = ginfo[l]
            cs = c_st[:, l, :]
            # gates per partition-half: [i|f|o|j] x 64
            sifo = work.tile([128, 192], F32, tag="sifo", bufs=3,
                             name=f"{rp}sifo{w}_{l}")
            nc.scalar.activation(sifo[:], g[:, 0:192], AF.Sigmoid)
            tj = work.tile([128, 64], F32, tag="tj", bufs=3,
                           name=f"{rp}tj{w}_{l}")
            nc.scalar.activation(tj[:], g[:, 192:256], AF.Tanh)
            t1 = work.tile([128, 64], F32, tag="t1", bufs=3,
                           name=f"{rp}t1{w}_{l}")
            nc.vector.tensor_mul(t1[:], sifo[:, 0:64], tj[:])
            t2 = work.tile([128, 64], F32, tag="t2", bufs=3,
                           name=f"{rp}t2{w}_{l}")
            nc.vector.tensor_mul(t2[:], cs, sifo[:, 64:128])
            nc.vector.tensor_add(cs, t1[:], t2[:])
            th = work.tile([128, 64], F32, tag="th", bufs=3,
                           name=f"{rp}th{w}_{l}")
            nc.scalar.activation(th[:], cs, AF.Tanh)
            hnew = work.tile([128, 64], F32, tag="hnew", bufs=3,
                             name=f"{rp}hnew{w}_{l}")
            nc.vector.tensor_mul(hnew[:], th[:], sifo[:, 128:192])
            if t == T - 1:
                nc.sync.dma_start(out=out[l, :, :], in_=hnew[:])
                if l == DEPTH - 1:
                    continue
            ht_ps = psum.tile([128, 64], F32, tag="htp", bufs=2,
                              name=f"{rp}htp{w}_{l}")
            nc.tensor.transpose(ht_ps[0:64, :], hnew[0:64, :],
                                identity=idf32_sb[:], tile_position=(0, 0))
            nc.tensor.transpose(ht_ps[64:128, :], hnew[64:128, :],
                                identity=idf32_hi[64:128, :],
                                tile_position=(64, 64))
            if rdma is None:
                hstg = work.tile([128, 64], BF16, tag="hstg", bufs=3,
                                 name=f"{rp}hstg{w}_{l}")
                nc.vector.tensor_copy(hstg[:], ht_ps[:])
                nc.sync.dma_start(out=cin[l, :, :], in_=hstg[:])
            else:
                nc.vector.tensor_copy(
                    rdma["hstg"][:, w % 2, l * 64:(l + 1) * 64], ht_ps[:])
        if w >= n_waves - 1:
            continue
        if rdma is not None:
            wp = w % 2
            for l in range(DEPTH):
                if not (0 <= w - l < T) and w < 2:
                    nc.vector.tensor_copy(
                        rdma["hstg"][:, wp, l * 64:(l + 1) * 64], ztile[:])
            hsrc = rdma["hstg"][:, wp, :]
            nc.vector.tensor_copy(
                rdma["hTr"][:, wp, 0, :, :],
                hsrc.rearrange("p (l b) -> p l b", l=DEPTH))
            for d in range(1, NCORE):
                nc.gpsimd.remote_dma_broadcast(
                    out_ap=rdma["hTr"][:, wp, d, :, :],
                    in_ap=hsrc,
                    remote_sem=rdma["rsem"],
                    local_sem=rdma["lsem"],
                    rdests=[(0, d) if k == d else None
                            for k in range(NCORE)],
                )
            nc.gpsimd.trigger_dma(count=None)
            rdma["sent_waves"] += 1
            continue
        cout = dram.tile([NCORE, DEPTH, 128, 64], BF16, tag="cout",
                         bufs=2, addr_space="Shared", name=f"{rp}cout{w}")
        nc.gpsimd.collective_compute(
            "AllGather", mybir.AluOpType.bypass,
            replica_groups=[list(range(NCORE))],
            ins=[cin.opt()], outs=[cout.opt()],
        )
        hT_g = work.tile([128, DEPTH, NCHUNK, 64], BF16, tag="hTg",
                         bufs=2, name=f"{rp}hTg{w}")
        for l in range(DEPTH):
            nc.sync.dma_start(
                out=hT_g[:, l, :, :],
                in_=cout[:, l, :, :].rearrange("r p b -> p r b"))
        hT_cur = hT_g
    ensure_phaseA(n_tiles)


# ---------------- host side ----------------

def _prep_inputs(tokens, emb, Ws, bs, T, comm="cc", arch="v1"):
    """Build per-core in_maps (numpy)."""
    n_tiles = (T * BATCH) // 128
    tok_flat = np.ascontiguousarray(tokens[:, :T].T).reshape(-1)  # t-major
    tok_sb = np.ascontiguousarray(tok_flat.reshape(n_tiles, 128).T).astype(np.int32)
    embt = emb.astype(bf16)
    idbf = np.eye(128, dtype=bf16)
    idf32 = np.eye(64, dtype=np.float32)
    ones = np.ones((1, 128), dtype=bf16)

    in_maps = []
    for k in range(NCORE):
        if arch == "v2":
            # [i0 f0 o0 j0 | i1 f1 o1 j1] by hidden half (64 each)
            cols = np.concatenate([
                np.arange(g * SIZE + k * 128 + h * 64,
                          g * SIZE + k * 128 + h * 64 + 64)
                for h in (0, 1) for g in (0, 2, 3, 1)
            ])
        else:
            cols = np.concatenate([
                np.arange(g * SIZE + k * 128, g * SIZE + (k + 1) * 128)
                for g in (0, 2, 3, 1)   # i, f, o, j
            ])
        Wsl = Ws[:, :, cols]                      # [3, 3072, 512] f32
        wx_c = np.ascontiguousarray(
            Wsl[:, :SIZE, :].reshape(DEPTH, NCHUNK, 128, GSL)
            .transpose(2, 1, 0, 3)).astype(bf16)  # [128, 8, 3, 512]
        wr_c = np.ascontiguousarray(
            Wsl[:, SIZE:, :].reshape(DEPTH, 2 * NCHUNK, 128, GSL)
            .transpose(2, 0, 1, 3)).astype(bf16)  # [128, 3, 16, 512]
        if comm == "rdma":
            # slot s of the gathered h buffer on core k holds the slice of
            # rank pinv[p[k] ^ s] (XOR-delta routing on physical NC index)
            p = PHYS_MAP
            pinv = [p.index(i) for i in range(NCORE)]
            perm = np.empty_like(wr_c)
            for s in range(NCHUNK):
                sig = pinv[p[k] ^ s]
                perm[:, :, s, :] = wr_c[:, :, sig, :]
                perm[:, :, NCHUNK + s, :] = wr_c[:, :, NCHUNK + sig, :]
            wr_c = perm
        bias_c = bs[:, cols][None].astype(bf16)   # [1, 3, 512]
        in_maps.append({
            "tok": tok_sb, "embt": embt, "wx": wx_c, "wr": wr_c,
            "bias": bias_c, "idbf": idbf, "idf32": idf32, "ones": ones,
        })
    return in_maps


_NC_CACHE = {}


def get_nc(T=256, interleave=True, reps=1, pack2=True, comm="cc", arch="v1"):
    key = (T, interleave, reps, pack2, comm, arch)
    if key not in _NC_CACHE:
        if arch == "v2":
            _NC_CACHE[key] = build_v2(T, reps=reps, comm=comm,
                                      interleave=interleave)
        else:
            _NC_CACHE[key] = build(T, interleave=interleave, reps=reps,
                                   pack2=pack2, comm=comm)
    return _NC_CACHE[key]


def run_on_hw(inputs, T, interleave=True, reps=1, pack2=True, comm="cc",
              arch="v1"):
    nc = get_nc(T, interleave, reps, pack2, comm, arch)
    in_maps = _prep_inputs(np.asarray(inputs["tokens"]), np.asarray(inputs["emb"]),
                           np.asarray(inputs["Ws"]), np.asarray(inputs["bs"]), T,
                           comm=comm, arch=arch)
    return run_bass_kernel_spmd(nc, in_maps, core_ids=list(range(NCORE)))


COMM_MODE = "cc"
ARCH = "v1"


def unshard_out(res, arch):
    full = np.empty((BATCH, DEPTH * SIZE), np.float32)
    for k in range(NCORE):
        if arch == "v2":
            o = np.asarray(res.results[k]["out"]).reshape(DEPTH, 2, 64, 64)
            for l in range(DEPTH):
                for h in (0, 1):
                    full[:, l * SIZE + k * 128 + h * 64:
                         l * SIZE + k * 128 + h * 64 + 64] = o[l, h]
        else:
            o = np.asarray(res.results[k]["out"]).reshape(DEPTH, BATCH, 128)
            for l in range(DEPTH):
                full[:, l * SIZE + k * 128:l * SIZE + (k + 1) * 128] = o[l]
    return full


def kernel(tokens, emb, Ws, bs):
    T = tokens.shape[1]
    nc = get_nc(T, comm=COMM_MODE, arch=ARCH)
    in_maps = _prep_inputs(np.asarray(tokens), np.asarray(emb),
                           np.asarray(Ws), np.asarray(bs), T, comm=COMM_MODE,
                           arch=ARCH)
    res = run_bass_kernel_spmd(nc, in_maps, core_ids=list(range(NCORE)))
    return unshard_out(res, ARCH)



# revision 10
# speedup vs baseline: 1.0899x; 1.0899x over previous
"""DeepLSTM (3-layer, skip-connection) Trainium2 kernel, 8-way tensor-parallel.

Sharding: core k owns hidden slice [k*128,(k+1)*128) of every layer (gate
columns i/f/o/j for that slice). Recurrent weights stay SBUF-resident.
Phase A precomputes Ax[l,t] = x_t @ Wx_l + b_l (gate-slice) into DRAM.
Phase B runs a skewed wavefront (layer l at t = w - l), one AllGather of
bf16 h-slices (transposed to [hid,batch]) per wave across the 8 cores.
"""
import numpy as np
import ml_dtypes

import concourse.bass as bass
import concourse.bacc as bacc
import concourse.mybir as mybir
import concourse.tile as tile
from concourse.bass import IndirectOffsetOnAxis
from concourse.bass_utils import run_bass_kernel_spmd

BF16 = mybir.dt.bfloat16
F32 = mybir.dt.float32
I32 = mybir.dt.int32

VOCAB, SIZE, DEPTH, BATCH = 32000, 1024, 3, 64
NCORE = 8
NCHUNK = SIZE // 128          # 8 K-chunks of the hidden dim
GSL = 4 * SIZE // NCORE       # 512 gate columns per core
AF = mybir.ActivationFunctionType

bf16 = ml_dtypes.bfloat16

# measured logical->physical NC map on this machine (probe_map.py); the
# remote_dma XOR-delta routing acts on PHYSICAL tpb indices
PHYS_MAP = [0, 1, 2, 3, 6, 7, 4, 5]


def build(T, interleave=True, reps=1, pack2=False,
          ag_mode="cc", phase_a=True, comm="cc"):
    """Build the SPMD bass program for T timesteps. Returns nc.

    reps > 1 repeats the whole computation back-to-back (same inputs,
    state re-zeroed) — used to measure pure HW exec time by differencing.
    """
    n_tiles = (T * BATCH) // 128          # token tiles of 128 (2 timesteps each)
    assert (T * BATCH) % 128 == 0
    n_waves = T + DEPTH - 1               # skewed wavefront

    nc = bacc.Bacc("TRN2", target_bir_lowering=False, debug=False,
                   num_devices=NCORE,
                   detect_race_conditions=(comm != "rdma"))

    # ---- DRAM I/O ----
    tok = nc.dram_tensor("tok", [128, n_tiles], I32, kind="ExternalInput")
    embt = nc.dram_tensor("embt", [VOCAB, SIZE], BF16, kind="ExternalInput")
    wx = nc.dram_tensor("wx", [128, NCHUNK, DEPTH, GSL], BF16, kind="ExternalInput")
    wr = nc.dram_tensor("wr", [128, DEPTH, 2 * NCHUNK, GSL], BF16, kind="ExternalInput")
    bias = nc.dram_tensor("bias", [1, DEPTH, GSL], BF16, kind="ExternalInput")
    idbf = nc.dram_tensor("idbf", [128, 128], BF16, kind="ExternalInput")
    idf32 = nc.dram_tensor("idf32", [64, 64], F32, kind="ExternalInput")
    ones = nc.dram_tensor("ones", [1, 128], BF16, kind="ExternalInput")
    out = nc.dram_tensor("out", [DEPTH, BATCH, 128], F32, kind="ExternalOutput")

    ax_dram = nc.dram_tensor("ax_dram", [DEPTH, n_tiles, 128, GSL], BF16)

    rdma = None
    if comm == "rdma":
        rdma = {
            "rsem": nc.alloc_semaphore("rdma_recv"),
            "lsem": nc.alloc_semaphore("rdma_sent"),
            "sent_waves": 0,   # cumulative across reps
            "patches": [],     # (mybir inst, rsem target) applied post-Tile
        }

    if rdma is not None:
        # all cores must be inside this NEFF before any remote SBUF write
        nc.gpsimd.bir_kernel_barrier_wait([list(range(NCORE))])

    with tile.TileContext(nc) as tc:
        with (
            tc.tile_pool(name="const", bufs=1) as constp,
            tc.tile_pool(name="state", bufs=1) as statep,
            tc.tile_pool(name="psum", bufs=1, space="PSUM") as psum,
            tc.tile_pool(name="work", bufs=2) as work,
            tc.tile_pool(name="dram", bufs=2, space="DRAM") as dram,
        ):
            # ---- resident SBUF constants/weights ----
            tok_sb = constp.tile([128, n_tiles], I32)
            nc.sync.dma_start(out=tok_sb[:], in_=tok[:])
            wx_sb = constp.tile([128, NCHUNK, DEPTH, GSL], BF16)
            nc.sync.dma_start(out=wx_sb[:], in_=wx[:])
            wr_sb = constp.tile([128, DEPTH, 2 * NCHUNK, GSL], BF16)
            nc.sync.dma_start(out=wr_sb[:], in_=wr[:])
            bias_sb = constp.tile([1, DEPTH, GSL], BF16)
            nc.sync.dma_start(out=bias_sb[:], in_=bias[:])
            idbf_sb = constp.tile([128, 128], BF16)
            nc.sync.dma_start(out=idbf_sb[:], in_=idbf[:])
            idf32_sb = constp.tile([64, 64], F32)
            nc.sync.dma_start(out=idf32_sb[:], in_=idf32[:])
            idf32_hi = constp.tile([128, 64], F32)
            nc.sync.dma_start(out=idf32_hi[64:128, :], in_=idf32[:])
            ones_sb = constp.tile([1, 128], BF16)
            nc.sync.dma_start(out=ones_sb[:], in_=ones[:])

            # ---- state ----
            c_st = statep.tile([128, DEPTH, 128], F32)  # cell state (own slice)
            hT_zero = statep.tile([128, DEPTH, NCHUNK, 64], BF16)
            nc.gpsimd.memset(hT_zero[:], 0.0)
            ztile = statep.tile([128, 64], BF16)
            nc.gpsimd.memset(ztile[:], 0.0)
            if rdma is not None:
                # double-buffered (by wave parity) send/recv SBUF buffers for
                # the hand-rolled all-gather via remote SBUF->SBUF DMA
                rdma["hstg"] = statep.tile([128, 2, DEPTH * 64], BF16,
                                           name="rdma_hstg")
                rdma["hTr"] = statep.tile([128, 2, NCORE, DEPTH, 64], BF16,
                                          name="rdma_hTr")

            for rep in range(reps):
                emit_rep(nc, tc, psum, work, dram, T, n_tiles, n_waves,
                         interleave, rep,
                         tok_sb, wx_sb, wr_sb, bias_sb, idbf_sb, idf32_sb,
                         ones_sb, c_st, hT_zero, ztile, ax_dram, embt, out,
                         pack2, idf32_hi, ag_mode, phase_a, rdma)

    if rdma is not None:
        # Inject arrival waits on every gathered-h consumer AFTER Tile
        # scheduling (the scheduler's single-core sim cannot model remote
        # semaphore increments and would deadlock on explicit wait_ge).
        sem_id = rdma["rsem"].num
        for ins, target in rdma["patches"]:
            wsync = mybir.SyncWait(sync_type="semaphore", id=sem_id,
                                   wait_mode="sem-ge-imm", wait_value=target,
                                   ant_name="rdma_recv")
            si = ins.sync_info
            if si is None:
                ins.sync_info = mybir.SyncInfo(on_wait=[wsync], on_update=[])
            else:
                si.on_wait = list(si.on_wait) + [wsync]
                ins.sync_info = si
    nc.compile()
    return nc


def emit_rep(nc, tc, psum, work, dram, T, n_tiles, n_waves, interleave, rep,
             tok_sb, wx_sb, wr_sb, bias_sb, idbf_sb, idf32_sb, ones_sb,
             c_st, hT_zero, ztile, ax_dram, embt, out, pack2=False,
             idf32_hi=None, ag_mode="cc", phase_a=True, rdma=None):
    if True:
        if True:
            nc.gpsimd.memset(c_st[:], 0.0)
            rp = f"r{rep}_"

            # ---- phase A: one token tile (128 tokens = 2 steps) ----
            def emit_phaseA_tile(i):
                xg = work.tile([128, SIZE], BF16, tag="xg", bufs=2,
                               name=f"{rp}xg{i}")
                nc.gpsimd.indirect_dma_start(
                    out=xg[:], out_offset=None, in_=embt[:, :],
                    in_offset=IndirectOffsetOnAxis(ap=tok_sb[:, i:i + 1], axis=0),
                )
                xt_ps = psum.tile([128, SIZE], BF16, tag="xt_ps", bufs=1,
                                  name=f"{rp}xtps{i}")
                for ch in range(NCHUNK):
                    nc.tensor.transpose(
                        out=xt_ps[:, ch * 128:(ch + 1) * 128],
                        in_=xg[:, ch * 128:(ch + 1) * 128],
                        identity=idbf_sb[:],
                    )
                xt = work.tile([128, SIZE], BF16, tag="xt", bufs=2, name=f"{rp}xt{i}")
                nc.vector.tensor_copy(xt[:], xt_ps[:])
                for l in range(DEPTH):
                    ax_ps = psum.tile([128, GSL], F32, tag="ax_ps", bufs=1,
                                      name=f"{rp}axps{i}_{l}")
                    for ch in range(NCHUNK):
                        nc.tensor.matmul(
                            ax_ps[:], lhsT=xt[:, ch * 128:(ch + 1) * 128],
                            rhs=wx_sb[:, ch, l, :],
                            start=(ch == 0), stop=False)
                    nc.tensor.matmul(ax_ps[:], lhsT=ones_sb[:, :],
                                     rhs=bias_sb[:, l, :], start=False, stop=True)
                    ax_sb = work.tile([128, GSL], BF16, tag="ax_st", bufs=3,
                                      name=f"{rp}axsb{i}_{l}")
                    nc.scalar.copy(ax_sb[:], ax_ps[:])
                    nc.sync.dma_start(out=ax_dram[l, i, :, :], in_=ax_sb[:])

            n_emitted = [0]

            def ensure_phaseA(upto):
                if not phase_a:
                    return  # timing-only variant: skip Ax precompute
                while n_emitted[0] < min(upto, n_tiles):
                    emit_phaseA_tile(n_emitted[0])
                    n_emitted[0] += 1

            if not interleave:
                ensure_phaseA(n_tiles)

            # ---- phase B: wavefront ----
            from concourse.bass import _add_dep_helper
            hT_cur = hT_zero
            for w in range(n_waves):
                if interleave:
                    # keep ~4 tiles of lookahead over the consuming wave
                    ensure_phaseA(w // 2 + 5)
                if rdma is None:
                    cin = dram.tile([DEPTH, 128, 64], BF16, tag="cin", bufs=2,
                                    name=f"{rp}cin{w}")
                    for l in range(DEPTH):
                        if not (0 <= w - l < T) and w < n_waves - 1:
                            # inactive layer: keep its gather slice zero/finite
                            nc.sync.dma_start(out=cin[l, :, :], in_=ztile[:])
                    def lhs_chunk(l, ch):
                        return hT_cur[:, l, ch, :]
                else:
                    wp = w % 2
                    if w == 0:
                        def lhs_chunk(l, ch):
                            return hT_zero[:, l, ch, :]
                    else:
                        def lhs_chunk(l, ch, _p=(w - 1) % 2):
                            return rdma["hTr"][:, _p, ch, l, :]
                actives = [l for l in range(DEPTH) if 0 <= w - l < T]
                ginfo = {}
                for l in actives:
                    t = w - l
                    ax_sb = work.tile([64, GSL], BF16, tag="ax_ld", bufs=6,
                                      name=f"{rp}axld{w}_{l}")
                    nc.sync.dma_start(
                        out=ax_sb[:],
                        in_=ax_dram[l, t // 2, (t % 2) * 64:(t % 2) * 64 + 64, :])
                    # pack2: layer 2 runs in PE array cols 64:128 concurrently
                    # with layer 1 in cols 0:64 (own psum bank, partitions
                    # 64:128); its whole elementwise chain stays on those
                    # partitions.
                    pb = 64 if (pack2 and l == 2) else 0
                    if pb:
                        g_full = psum.tile([128, GSL], F32, tag="g2", bufs=2,
                                           name=f"{rp}g{w}_{l}")
                        gv = g_full[64:128, :]
                        tp = (0, 64)
                    else:
                        gbufs = 2 if pack2 else 3
                        g_full = psum.tile([64, GSL], F32, tag="g", bufs=gbufs,
                                           name=f"{rp}g{w}_{l}")
                        gv = g_full[:]
                        tp = None
                    # inject Ax (+bias) via identity matmul, clears the bank
                    nc.tensor.matmul(gv, lhsT=idbf_sb[:64, :64],
                                     rhs=ax_sb[:],
                                     start=True, stop=False,
                                     tile_position=tp)
                    mms = [(lhs_chunk(l, ch), wr_sb[:, l, NCHUNK + ch, :])
                           for ch in range(NCHUNK)]
                    if l > 0:                  # prev-layer skip input @ Wp
                        mms += [(lhs_chunk(l - 1, ch), wr_sb[:, l, ch, :])
                                for ch in range(NCHUNK)]
                    ginfo[l] = (t, pb, gv, tp, mms)

                def emit_stream(l, j):
                    t, pb, gv, tp, mms = ginfo[l]
                    lhsT, rhs = mms[j]
                    bi = nc.tensor.matmul(gv, lhsT=lhsT, rhs=rhs, start=False,
                                          stop=(j == len(mms) - 1),
                                          tile_position=tp)
                    if rdma is not None and w > 0:
                        rdma["patches"].append(
                            (bi.ins, 14 * rdma["sent_waves"]))

                if pack2:
                    paired = [l for l in (1, 2) if l in ginfo]
                    plen = max((len(ginfo[l][4]) for l in paired), default=0)
                    for j in range(plen):
                        for l in paired:
                            if j < len(ginfo[l][4]):
                                emit_stream(l, j)
                    if 0 in ginfo:
                        for j in range(len(ginfo[0][4])):
                            emit_stream(0, j)
                else:
                    for l in actives:
                        for j in range(len(ginfo[l][4])):
                            emit_stream(l, j)

                for l in actives:
                    t, pb, gv, tp, mms = ginfo[l]
                    sl = slice(pb, pb + 64)
                    cs = c_st[sl, l, :]
                    # gates: layout [i|f|o|j], 128 each
                    sifo = work.tile([128, 384], F32, tag="sifo", bufs=3,
                                     name=f"{rp}sifo{w}_{l}")
                    nc.scalar.activation(sifo[sl, :], gv[:, 0:384], AF.Sigmoid)
                    tj = work.tile([128, 128], F32, tag="tj", bufs=3,
                                   name=f"{rp}tj{w}_{l}")
                    nc.scalar.activation(tj[sl, :], gv[:, 384:512], AF.Tanh)
                    t1 = work.tile([128, 128], F32, tag="t1", bufs=3,
                                   name=f"{rp}t1{w}_{l}")
                    nc.vector.tensor_mul(t1[sl, :], sifo[sl, 0:128], tj[sl, :])
                    t2 = work.tile([128, 128], F32, tag="t2", bufs=3,
                                   name=f"{rp}t2{w}_{l}")
                    nc.vector.tensor_mul(t2[sl, :], cs, sifo[sl, 128:256])
                    nc.vector.tensor_add(cs, t1[sl, :], t2[sl, :])
                    th = work.tile([128, 128], F32, tag="th", bufs=3,
                                   name=f"{rp}th{w}_{l}")
                    nc.scalar.activation(th[sl, :], cs, AF.Tanh)
                    hnew = work.tile([128, 128], F32, tag="hnew", bufs=3,
                                     name=f"{rp}hnew{w}_{l}")
                    nc.vector.tensor_mul(hnew[sl, :], th[sl, :], sifo[sl, 256:384])
                    if t == T - 1:
                        nc.sync.dma_start(out=out[l, :, :], in_=hnew[sl, :])
                        if l == DEPTH - 1:
                            continue  # top layer: no future consumer
                    ht_ps = psum.tile([128, 64], F32, tag="htp", bufs=2,
                                      name=f"{rp}htp{w}_{l}")
                    ident = idf32_hi[64:128, :] if pb else idf32_sb[:]
                    nc.tensor.transpose(ht_ps[:], hnew[sl, :], identity=ident,
                                        tile_position=(pb, 0))
                    if rdma is None:
                        hstg = work.tile([128, 64], BF16, tag="hstg", bufs=3,
                                         name=f"{rp}hstg{w}_{l}")
                        nc.vector.tensor_copy(hstg[:], ht_ps[:])
                        nc.sync.dma_start(out=cin[l, :, :], in_=hstg[:])
                    else:
                        nc.vector.tensor_copy(
                            rdma["hstg"][:, wp, l * 64:(l + 1) * 64], ht_ps[:])
                if w >= n_waves - 1:
                    continue  # last wave: no gather needed
                if rdma is not None:
                    # zero inactive-layer slices of the send buffer (finite +
                    # correct zeros for the t<0 self-inputs of early waves)
                    for l in range(DEPTH):
                        if not (0 <= w - l < T) and w < 2:
                            nc.vector.tensor_copy(
                                rdma["hstg"][:, wp, l * 64:(l + 1) * 64],
                                ztile[:])
                    hsrc = rdma["hstg"][:, wp, :]
                    # own slice lands in slot 0 (r ^ 0 = r)
                    nc.vector.tensor_copy(
                        rdma["hTr"][:, wp, 0, :, :],
                        hsrc.rearrange("p (l b) -> p l b", l=DEPTH))
                    for d in range(1, NCORE):
                        nc.gpsimd.remote_dma_broadcast(
                            out_ap=rdma["hTr"][:, wp, d, :, :],
                            in_ap=hsrc,
                            remote_sem=rdma["rsem"],
                            local_sem=rdma["lsem"],
                            rdests=[(0, d) if k == d else None
                                    for k in range(NCORE)],
                        )
                    nc.gpsimd.trigger_dma(count=None)
                    rdma["sent_waves"] += 1
                    continue
                cout = dram.tile([NCORE, DEPTH, 128, 64], BF16, tag="cout",
                                 bufs=2, addr_space="Shared", name=f"{rp}cout{w}")
                if ag_mode == "cc":
                    nc.gpsimd.collective_compute(
                        "AllGather", mybir.AluOpType.bypass,
                        replica_groups=[list(range(NCORE))],
                        ins=[cin.opt()], outs=[cout.opt()],
                    )
                else:  # timing-only variant: local copy instead of AllGather
                    nc.sync.dma_start(out=cout[0, :, :, :], in_=cin[:, :, :])
                hT_g = work.tile([128, DEPTH, NCHUNK, 64], BF16, tag="hTg",
                                 bufs=2, name=f"{rp}hTg{w}")
                for l in range(DEPTH):
                    nc.sync.dma_start(
                        out=hT_g[:, l, :, :],
                        in_=cout[:, l, :, :].rearrange("r p b -> p r b"))
                hT_cur = hT_g
            ensure_phaseA(n_tiles)  # no-op normally


# ---------------- v2: split-gate dual-column-group wavefront ----------------

def build_v2(T, reps=1, comm="rdma", interleave=True, phase_a=True):
    """v2: each layer's 512 gate cols (reordered [i0 f0 o0 j0 | i1 f1 o1 j1]
    by hidden half) are computed by two concurrent PE column-group streams
    (batch=64 rows each, N=256), landing in one [128, 256] PSUM tile
    (partitions = batch x half). Elementwise then runs full-width [128, *].
    """
    n_tiles = (T * BATCH) // 128
    assert (T * BATCH) % 128 == 0
    n_waves = T + DEPTH - 1

    nc = bacc.Bacc("TRN2", target_bir_lowering=False, debug=False,
                   num_devices=NCORE,
                   detect_race_conditions=(comm != "rdma"))

    tok = nc.dram_tensor("tok", [128, n_tiles], I32, kind="ExternalInput")
    embt = nc.dram_tensor("embt", [VOCAB, SIZE], BF16, kind="ExternalInput")
    wx = nc.dram_tensor("wx", [128, NCHUNK, DEPTH, GSL], BF16, kind="ExternalInput")
    wr = nc.dram_tensor("wr", [128, DEPTH, 2 * NCHUNK, GSL], BF16, kind="ExternalInput")
    bias = nc.dram_tensor("bias", [1, DEPTH, GSL], BF16, kind="ExternalInput")
    idbf = nc.dram_tensor("idbf", [128, 128], BF16, kind="ExternalInput")
    idf32 = nc.dram_tensor("idf32", [64, 64], F32, kind="ExternalInput")
    ones = nc.dram_tensor("ones", [1, 128], BF16, kind="ExternalInput")
    out = nc.dram_tensor("out", [DEPTH, 128, 64], F32, kind="ExternalOutput")

    ax_dram = nc.dram_tensor("ax_dram", [DEPTH, n_tiles, 128, GSL], BF16)

    rdma = None
    if comm == "rdma":
        rdma = {
            "rsem": nc.alloc_semaphore("rdma_recv"),
            "lsem": nc.alloc_semaphore("rdma_sent"),
            "sent_waves": 0,
            "patches": [],
        }
        nc.gpsimd.bir_kernel_barrier_wait([list(range(NCORE))])

    with tile.TileContext(nc) as tc:
        with (
            tc.tile_pool(name="const", bufs=1) as constp,
            tc.tile_pool(name="state", bufs=1) as statep,
            tc.tile_pool(name="psum", bufs=1, space="PSUM") as psum,
            tc.tile_pool(name="work", bufs=2) as work,
            tc.tile_pool(name="dram", bufs=2, space="DRAM") as dram,
        ):
            tok_sb = constp.tile([128, n_tiles], I32)
            nc.sync.dma_start(out=tok_sb[:], in_=tok[:])
            wx_sb = constp.tile([128, NCHUNK, DEPTH, GSL], BF16)
            nc.sync.dma_start(out=wx_sb[:], in_=wx[:])
            wr_sb = constp.tile([128, DEPTH, 2 * NCHUNK, GSL], BF16)
            nc.sync.dma_start(out=wr_sb[:], in_=wr[:])
            bias_sb = constp.tile([1, DEPTH, GSL], BF16)
            nc.sync.dma_start(out=bias_sb[:], in_=bias[:])
            idbf_sb = constp.tile([128, 128], BF16)
            nc.sync.dma_start(out=idbf_sb[:], in_=idbf[:])
            idf32_sb = constp.tile([64, 64], F32)
            nc.sync.dma_start(out=idf32_sb[:], in_=idf32[:])
            idf32_hi = constp.tile([128, 64], F32)
            nc.sync.dma_start(out=idf32_hi[64:128, :], in_=idf32[:])
            ones_sb = constp.tile([1, 128], BF16)
            nc.sync.dma_start(out=ones_sb[:], in_=ones[:])

            # state: partitions = batch (0:64 -> hid half 0, 64:128 -> half 1)
            c_st = statep.tile([128, DEPTH, 64], F32)
            hT_zero = statep.tile([128, DEPTH, NCHUNK, 64], BF16)
            nc.gpsimd.memset(hT_zero[:], 0.0)
            ztile = statep.tile([128, 64], BF16)
            nc.gpsimd.memset(ztile[:], 0.0)
            if rdma is not None:
                rdma["hstg"] = statep.tile([128, 2, DEPTH * 64], BF16,
                                           name="rdma_hstg")
                rdma["hTr"] = statep.tile([128, 2, NCORE, DEPTH, 64], BF16,
                                          name="rdma_hTr")

            for rep in range(reps):
                emit_rep_v2(nc, tc, psum, work, dram, T, n_tiles, n_waves,
                            interleave, rep,
                            tok_sb, wx_sb, wr_sb, bias_sb, idbf_sb, idf32_sb,
                            ones_sb, c_st, hT_zero, ztile, ax_dram, embt, out,
                            idf32_hi, phase_a, rdma)

    if rdma is not None:
        sem_id = rdma["rsem"].num
        for ins, target in rdma["patches"]:
            wsync = mybir.SyncWait(sync_type="semaphore", id=sem_id,
                                   wait_mode="sem-ge-imm", wait_value=target,
                                   ant_name="rdma_recv")
            si = ins.sync_info
            if si is None:
                ins.sync_info = mybir.SyncInfo(on_wait=[wsync], on_update=[])
            else:
                si.on_wait = list(si.on_wait) + [wsync]
                ins.sync_info = si
    nc.compile()
    return nc


def emit_rep_v2(nc, tc, psum, work, dram, T, n_tiles, n_waves, interleave, rep,
                tok_sb, wx_sb, wr_sb, bias_sb, idbf_sb, idf32_sb, ones_sb,
                c_st, hT_zero, ztile, ax_dram, embt, out,
                idf32_hi, phase_a=True, rdma=None):
    nc.gpsimd.memset(c_st[:], 0.0)
    rp = f"r{rep}_"

    # ---- phase A (identical mechanics to v1; gate order is host-side) ----
    def emit_phaseA_tile(i):
        xg = work.tile([128, SIZE], BF16, tag="xg", bufs=2, name=f"{rp}xg{i}")
        nc.gpsimd.indirect_dma_start(
            out=xg[:], out_offset=None, in_=embt[:, :],
            in_offset=IndirectOffsetOnAxis(ap=tok_sb[:, i:i + 1], axis=0),
        )
        xt_ps = psum.tile([128, SIZE], BF16, tag="xt_ps", bufs=1,
                          name=f"{rp}xtps{i}")
        for ch in range(NCHUNK):
            nc.tensor.transpose(
                out=xt_ps[:, ch * 128:(ch + 1) * 128],
                in_=xg[:, ch * 128:(ch + 1) * 128],
                identity=idbf_sb[:],
            )
        xt = work.tile([128, SIZE], BF16, tag="xt", bufs=2, name=f"{rp}xt{i}")
        nc.vector.tensor_copy(xt[:], xt_ps[:])
        for l in range(DEPTH):
            ax_ps = psum.tile([128, GSL], F32, tag="ax_ps", bufs=1,
                              name=f"{rp}axps{i}_{l}")
            for ch in range(NCHUNK):
                nc.tensor.matmul(
                    ax_ps[:], lhsT=xt[:, ch * 128:(ch + 1) * 128],
                    rhs=wx_sb[:, ch, l, :],
                    start=(ch == 0), stop=False)
            nc.tensor.matmul(ax_ps[:], lhsT=ones_sb[:, :],
                             rhs=bias_sb[:, l, :], start=False, stop=True)
            ax_sb = work.tile([128, GSL], BF16, tag="ax_st", bufs=3,
                              name=f"{rp}axsb{i}_{l}")
            nc.scalar.copy(ax_sb[:], ax_ps[:])
            nc.sync.dma_start(out=ax_dram[l, i, :, :], in_=ax_sb[:])

    n_emitted = [0]

    def ensure_phaseA(upto):
        if not phase_a:
            return
        while n_emitted[0] < min(upto, n_tiles):
            emit_phaseA_tile(n_emitted[0])
            n_emitted[0] += 1

    if not interleave:
        ensure_phaseA(n_tiles)

    # ---- phase B ----
    hT_cur = hT_zero
    for w in range(n_waves):
        if interleave:
            ensure_phaseA(w // 2 + 5)
        if rdma is None:
            cin = dram.tile([DEPTH, 128, 64], BF16, tag="cin", bufs=2,
                            name=f"{rp}cin{w}")
            for l in range(DEPTH):
                if not (0 <= w - l < T) and w < n_waves - 1:
                    nc.sync.dma_start(out=cin[l, :, :], in_=ztile[:])
            def lhs_chunk(l, ch):
                return hT_cur[:, l, ch, :]
        else:
            wp = w % 2
            if w == 0:
                def lhs_chunk(l, ch):
                    return hT_zero[:, l, ch, :]
            else:
                def lhs_chunk(l, ch, _p=(w - 1) % 2):
                    return rdma["hTr"][:, _p, ch, l, :]
        actives = [l for l in range(DEPTH) if 0 <= w - l < T]
        ginfo = {}
        for l in actives:
            t = w - l
            ax_sb = work.tile([64, GSL], BF16, tag="ax_ld", bufs=6,
                              name=f"{rp}axld{w}_{l}")
            nc.sync.dma_start(
                out=ax_sb[:],
                in_=ax_dram[l, t // 2, (t % 2) * 64:(t % 2) * 64 + 64, :])
            # one [128, 256] psum tile per layer; padded to a full bank so
            # no two accumulation groups share a bank (has_written clear is
            # bank-wide). Stream A -> partitions 0:64 (cols 0:256 of the
            # reordered gates), stream B -> partitions 64:128 (cols 256:512).
            g = psum.tile([128, 256], F32, tag="g", bufs=3,
                          padded_shape=[128, 512], name=f"{rp}g{w}_{l}")
            # inject Ax+bias via identity matmuls (clears both halves)
            nc.tensor.matmul(g[0:64, :], lhsT=idbf_sb[:64, :64],
                             rhs=ax_sb[:, 0:256], start=True, stop=False,
                             tile_position=(0, 0))
            nc.tensor.matmul(g[64:128, :], lhsT=idbf_sb[:64, :64],
                             rhs=ax_sb[:, 256:512], start=True, stop=False,
                             tile_position=(0, 64), skip_group_check=True)
            mms = [(lhs_chunk(l, ch), wr_sb[:, l, NCHUNK + ch, :])
                   for ch in range(NCHUNK)]
            if l > 0:
                mms += [(lhs_chunk(l - 1, ch), wr_sb[:, l, ch, :])
                        for ch in range(NCHUNK)]
            ginfo[l] = (t, g, mms)

        for l in actives:
            t, g, mms = ginfo[l]
            for j, (lhsT, rhs) in enumerate(mms):
                last = (j == len(mms) - 1)
                biA = nc.tensor.matmul(g[0:64, :], lhsT=lhsT, rhs=rhs[..., 0:256],
                                       start=False, stop=last,
                                       tile_position=(0, 0))
                biB = nc.tensor.matmul(g[64:128, :], lhsT=lhsT,
                                       rhs=rhs[..., 256:512],
                                       start=False, stop=last,
                                       tile_position=(0, 64),
                                       skip_group_check=True)
                if rdma is not None and w > 0:
                    tgt = 14 * rdma["sent_waves"]
                    rdma["patches"].append((biA.ins, tgt))
                    rdma["patches"].append((biB.ins, tgt))

        for l in actives:
            t, g, mms = ginfo[l]
            cs = c_st[:, l, :]
            # gates per partition-half: [i|f|o|j] x 64
            sifo = work.tile([128, 192], F32, tag="sifo", bufs=3,
                             name=f"{rp}sifo{w}_{l}")
            nc.scalar.activation(sifo[:], g[:, 0:192], AF.Sigmoid)
            tj = work.tile([128, 64], F32, tag="tj", bufs=3,
                           name=f"{rp}tj{w}_{l}")
            nc.scalar.activation(tj[:], g[:, 192:256], AF.Tanh)
            t1 = work.tile([128, 64], F32, tag="t1", bufs=3,
                           name=f"{rp}t1{w}_{l}")
            nc.vector.tensor_mul(t1[:], sifo[:, 0:64], tj[:])
            t2 = work.tile([128, 64], F32, tag="t2", bufs=3,
                           name=f"{rp}t2{w}_{l}")
            nc.vector.tensor_mul(t2[:], cs, sifo[:, 64:128])
            nc.vector.tensor_add(cs, t1[:], t2[:])
            th = work.tile([128, 64], F32, tag="th", bufs=3,
                           name=f"{rp}th{w}_{l}")
            nc.scalar.activation(th[:], cs, AF.Tanh)
            hnew = work.tile([128, 64], BF16, tag="hnew", bufs=3,
                             name=f"{rp}hnew{w}_{l}")
            nc.vector.tensor_mul(hnew[:], th[:], sifo[:, 128:192])
            if t == T - 1:
                # cast bf16 -> f32 during the output DMA (SWDGE)
                nc.gpsimd.dma_start(out=out[l, :, :], in_=hnew[:])
                if l == DEPTH - 1:
                    continue
            # transpose both halves via normal identity matmuls (transpose-
            # mode MMs must output at PSUM partition 0; these need not)
            ht_ps = psum.tile([128, 64], F32, tag="htp", bufs=2,
                              name=f"{rp}htp{w}_{l}")
            nc.tensor.matmul(ht_ps[0:64, :], lhsT=hnew[0:64, :],
                             rhs=idbf_sb[0:64, 0:64], start=True, stop=True,
                             tile_position=(0, 0))
            nc.tensor.matmul(ht_ps[64:128, :], lhsT=hnew[64:128, :],
                             rhs=idbf_sb[64:128, 64:128], start=True, stop=True,
                             tile_position=(64, 64))
            if rdma is None:
                hstg = work.tile([128, 64], BF16, tag="hstg", bufs=3,
                                 name=f"{rp}hstg{w}_{l}")
                nc.vector.tensor_copy(hstg[:], ht_ps[:])
                nc.sync.dma_start(out=cin[l, :, :], in_=hstg[:])
            else:
                nc.vector.tensor_copy(
                    rdma["hstg"][:, w % 2, l * 64:(l + 1) * 64], ht_ps[:])
        if w >= n_waves - 1:
            continue
        if rdma is not None:
            wp = w % 2
            for l in range(DEPTH):
                if not (0 <= w - l < T) and w < 2:
                    nc.vector.tensor_copy(
                        rdma["hstg"][:, wp, l * 64:(l + 1) * 64], ztile[:])
            hsrc = rdma["hstg"][:, wp, :]
            nc.vector.tensor_copy(
                rdma["hTr"][:, wp, 0, :, :],
                hsrc.rearrange("p (l b) -> p l b", l=DEPTH))
            for d in range(1, NCORE):
                nc.gpsimd.remote_dma_broadcast(
                    out_ap=rdma["hTr"][:, wp, d, :, :],
                    in_ap=hsrc,
                    remote_sem=rdma["rsem"],
                    local_sem=rdma["lsem"],
                    rdests=[(0, d) if k == d else None
                            for k in range(NCORE)],
                )
            nc.gpsimd.trigger_dma(count=None)
            rdma["sent_waves"] += 1
            continue
        cout = dram.tile([NCORE, DEPTH, 128, 64], BF16, tag="cout",
                         bufs=2, addr_space="Shared", name=f"{rp}cout{w}")
        nc.gpsimd.collective_compute(
            "AllGather", mybir.AluOpType.bypass,
            replica_groups=[list(range(NCORE))],
            ins=[cin.opt()], outs=[cout.opt()],
        )
        hT_g = work.tile([128, DEPTH, NCHUNK, 64], BF16, tag="hTg",
                         bufs=2, name=f"{rp}hTg{w}")
        for l in range(DEPTH):
            nc.sync.dma_start(
                out=hT_g[:, l, :, :],
                in_=cout[:, l, :, :].rearrange("r p b -> p r b"))
        hT_cur = hT_g
    ensure_phaseA(n_tiles)


# ---------------- host side ----------------

def _prep_inputs(tokens, emb, Ws, bs, T, comm="cc", arch="v1"):
    """Build per-core in_maps (numpy)."""
    n_tiles = (T * BATCH) // 128
    tok_flat = np.ascontiguousarray(tokens[:, :T].T).reshape(-1)  # t-major
    tok_sb = np.ascontiguousarray(tok_flat.reshape(n_tiles, 128).T).astype(np.int32)
    embt = emb.astype(bf16)
    idbf = np.eye(128, dtype=bf16)
    idf32 = np.eye(64, dtype=np.float32)
    ones = np.ones((1, 128), dtype=bf16)

    in_maps = []
    for k in range(NCORE):
        if arch == "v2":
            # [i0 f0 o0 j0 | i1 f1 o1 j1] by hidden half (64 each)
            cols = np.concatenate([
                np.arange(g * SIZE + k * 128 + h * 64,
                          g * SIZE + k * 128 + h * 64 + 64)
                for h in (0, 1) for g in (0, 2, 3, 1)
            ])
        else:
            cols = np.concatenate([
                np.arange(g * SIZE + k * 128, g * SIZE + (k + 1) * 128)
                for g in (0, 2, 3, 1)   # i, f, o, j
            ])
        Wsl = Ws[:, :, cols]                      # [3, 3072, 512] f32
        wx_c = np.ascontiguousarray(
            Wsl[:, :SIZE, :].reshape(DEPTH, NCHUNK, 128, GSL)
            .transpose(2, 1, 0, 3)).astype(bf16)  # [128, 8, 3, 512]
        wr_c = np.ascontiguousarray(
            Wsl[:, SIZE:, :].reshape(DEPTH, 2 * NCHUNK, 128, GSL)
            .transpose(2, 0, 1, 3)).astype(bf16)  # [128, 3, 16, 512]
        if comm == "rdma":
            # slot s of the gathered h buffer on core k holds the slice of
            # rank pinv[p[k] ^ s] (XOR-delta routing on physical NC index)
            p = PHYS_MAP
            pinv = [p.index(i) for i in range(NCORE)]
            perm = np.empty_like(wr_c)
            for s in range(NCHUNK):
                sig = pinv[p[k] ^ s]
                perm[:, :, s, :] = wr_c[:, :, sig, :]
                perm[:, :, NCHUNK + s, :] = wr_c[:, :, NCHUNK + sig, :]
            wr_c = perm
        bias_c = bs[:, cols][None].astype(bf16)   # [1, 3, 512]
        in_maps.append({
            "tok": tok_sb, "embt": embt, "wx": wx_c, "wr": wr_c,
            "bias": bias_c, "idbf": idbf, "idf32": idf32, "ones": ones,
        })
    return in_maps


_NC_CACHE = {}


def get_nc(T=256, interleave=True, reps=1, pack2=True, comm="cc", arch="v1"):
    key = (T, interleave, reps, pack2, comm, arch)
    if key not in _NC_CACHE:
        if arch == "v2":
            _NC_CACHE[key] = build_v2(T, reps=reps, comm=comm,
                                      interleave=interleave)
        else:
            _NC_CACHE[key] = build(T, interleave=interleave, reps=reps,
                                   pack2=pack2, comm=comm)
    return _NC_CACHE[key]


def run_on_hw(inputs, T, interleave=True, reps=1, pack2=True, comm="cc",
              arch="v1"):
    nc = get_nc(T, interleave, reps, pack2, comm, arch)
    in_maps = _prep_inputs(np.asarray(inputs["tokens"]), np.asarray(inputs["emb"]),
                           np.asarray(inputs["Ws"]), np.asarray(inputs["bs"]), T,
                           comm=comm, arch=arch)
    return run_bass_kernel_spmd(nc, in_maps, core_ids=list(range(NCORE)))


COMM_MODE = "cc"
ARCH = "v1"


def unshard_out(res, arch):
    full = np.empty((BATCH, DEPTH * SIZE), np.float32)
    for k in range(NCORE):
        if arch == "v2":
            o = np.asarray(res.results[k]["out"]).reshape(DEPTH, 2, 64, 64)
            for l in range(DEPTH):
                for h in (0, 1):
                    full[:, l * SIZE + k * 128 + h * 64:
                         l * SIZE + k * 128 + h * 64 + 64] = o[l, h]
        else:
            o = np.asarray(res.results[k]["out"]).reshape(DEPTH, BATCH, 128)
            for l in range(DEPTH):
                full[:, l * SIZE + k * 128:l * SIZE + (k + 1) * 128] = o[l]
    return full


def kernel(tokens, emb, Ws, bs):
    T = tokens.shape[1]
    nc = get_nc(T, comm=COMM_MODE, arch=ARCH)
    in_maps = _prep_inputs(np.asarray(tokens), np.asarray(emb),
                           np.asarray(Ws), np.asarray(bs), T, comm=COMM_MODE,
                           arch=ARCH)
    res = run_bass_kernel_spmd(nc, in_maps, core_ids=list(range(NCORE)))
    return unshard_out(res, ARCH)

